# revision 25
# baseline (speedup 1.0000x reference)
"""AttnBlock (GroupNorm + 1x1-conv spatial self-attention + residual) on 8 TRN2 cores.

Sharding: core = (batch b, pixel-quarter q). Each core computes the full
GroupNorm stats for its batch, then attention output rows for its 1024
pixels (i-dim), attending over all 4096 pixels (j-dim). Inputs are
host-rotated per core so the compiled program is identical (SPMD).

Algebraic folds (host side, fp64):
  - scores = hn^T (Wk^T Wq / sqrt(c)) hn  ->  one projection G = Wkq @ hn
  - bk cancels in softmax (constant along j); bq kept via bg = Wk^T bq_s
  - Wo @ Wv folded into one matrix; bo' = Wo @ bv + bo added at the end
  - softmax max-subtraction skipped (scores ~ N(0, 1/9); exp is safe)
  - 1/rowsum applied after the AV matmul, broadcast across partitions by
    a K=1 outer-product matmul.

fp8 fast path (on-chip):
  - all four big GEMMs (G, voT, scores, AV) are fp8e4 DoubleRow matmuls
    (2 fp8 weights/PE cell, K=256/instruction, 2x the f32r row rate).
  - GroupNorm is folded into the weights instead of materializing hn:
    with hn = D x + s 1^T (D=diag(gamma*rstd), s the shift),
      scores == x^T G2 + (term const over j, cancels in softmax)
      G2 = D (W2^T x + Wkq^T s + bg),  W2 = D Wkq
      voT == x^T (D Wov) + 1 (s^T Wov)
    so raw fp8 x feeds every GEMM; D scales the fp8 weights in place
    (per-partition), and the s-terms are tiny on-chip matvecs.
  - weights pre-scaled on host (Wkq x256, Wov x16) to keep fp8 out of
    subnormals; inverses fold into the exp activation scale (1/256) and
    the rowsum reduction constant (16).
  - GroupNorm stats split across engines: DVE bn_stats (chunks 0,3),
    ACT Square/Identity+accum (chunk 1), GPSIMD stt+accum (chunk 2).
  - x DMA'd as fp8 chunks on 4 parallel queues; residual stays fp32 (xt).
"""

import numpy as np
import ml_dtypes

B, C, H, W = 2, 512, 64, 64
HW = H * W               # 4096
P = 128                  # partitions
NCK = C // P             # 4 channel chunks
NKP = NCK // 2           # 2 chunk-pairs (DoubleRow K=256)
QPIX = HW // 4           # 1024 pixels per core
NIB = 2                  # i-blocks of 512 per core
IBS = QPIX // NIB        # 512
NJT = HW // P            # 32 j-tiles
NJP = NJT // 2           # 16 j-tile pairs
NSUB = HW // 512         # 8 bn_stats subgroups
EPS = 1e-6
WKQ_S = 256.0            # host pre-scale on Wkq (folded out in exp scale)
WOV_S = 16.0             # host pre-scale on Wov (folded out in rowsum const)
SHF_S = 256.0            # on-chip pre-scale on the GN shift before fp8

_CACHE = {}


def _build_nc():
    import concourse.bass as bass
    import concourse.tile as tile
    from concourse import bacc, mybir
    from contextlib import ExitStack

    f32 = mybir.dt.float32
    f32r = mybir.dt.float32r
    f8 = mybir.dt.float8e4
    AF = mybir.ActivationFunctionType
    OP = mybir.AluOpType
    DR = mybir.MatmulPerfMode.DoubleRow

    nc = bacc.Bacc("TRN2", target_bir_lowering=False, debug=False,
                   enable_asserts=False, num_devices=8)

    x_d = nc.dram_tensor("x", [C, HW], f8, kind="ExternalInput")
    wkqt_d = nc.dram_tensor("wkqt", [P, NKP, 2, C], f8, kind="ExternalInput")
    wovt_d = nc.dram_tensor("wovt", [P, NKP, 2, C], f8, kind="ExternalInput")
    pvec_d = nc.dram_tensor("pvec", [NCK, P, 3], f32, kind="ExternalInput")
    xt_d = nc.dram_tensor("xt", [QPIX, C], f32, kind="ExternalInput")
    out_d = nc.dram_tensor("out", [QPIX, C], f32, kind="ExternalOutput")

    # group-aggregation selectors (constant): 32 groups of 16 channels; a
    # channel chunk of 128 holds 8 whole groups.
    sel_np = np.zeros((P, 8), np.float32)
    for p in range(P):
        sel_np[p, p // 16] = 1.0 / 16.0
    selt_np = np.zeros((8, P), np.float32)
    for p in range(P):
        selt_np[p // 16, p] = 1.0
    sel_d = nc.inline_tensor(sel_np, "selc")
    selt_d = nc.inline_tensor(selt_np, "seltc")

    x_r = x_d.ap().rearrange("(kp two p) n -> kp two p n", p=P, two=2)
    out_r = out_d.ap().rearrange("(g p) o -> g p o", p=P)

    with tile.TileContext(nc) as tc, ExitStack() as ctx:
        perm = ctx.enter_context(tc.tile_pool(name="perm", bufs=1))
        gnp = ctx.enter_context(tc.tile_pool(name="gnwork", bufs=2))

        # x chunks first: fp8, four parallel DMA queues, chunk-major so
        # per-chunk stats can chase the stream.
        x8 = perm.tile([P, NKP, 2, HW], f8, name="x8", tag="x8")
        # halves of each chunk ride different queues so no engine starves
        qmap = {(0, 0): nc.sync, (0, 1): nc.scalar,
                (1, 0): nc.gpsimd, (1, 1): nc.sync,
                (2, 0): nc.scalar, (2, 1): nc.gpsimd,
                (3, 0): nc.sync, (3, 1): nc.scalar}
        for ck in range(NCK):
            for h in range(2):
                sl = slice(h * (HW // 2), (h + 1) * (HW // 2))
                qmap[(ck, h)].dma_start(out=x8[:, ck // 2, ck % 2, sl],
                                        in_=x_r[ck // 2, ck % 2][:, sl])

        # constants
        sel_sb = perm.tile([P, 8], f32, name="sel", tag="sel")
        nc.gpsimd.dma_start(out=sel_sb, in_=sel_d.ap())
        selt_sb = perm.tile([8, P], f32, name="selt", tag="selt")
        nc.gpsimd.dma_start(out=selt_sb, in_=selt_d.ap())
        # rowsum reduction constant: WOV_S folds the vot pre-scale back out
        sixt_sb = perm.tile([P, 1], f32, name="sixt", tag="sixt")
        nc.vector.memset(sixt_sb, WOV_S)
        ones1f = perm.tile([1, P], f32, name="ones1f", tag="ones1f")
        nc.vector.memset(ones1f, 1.0)
        ones1_sb = perm.tile([1, P], f32r, name="ones1", tag="ones1")
        nc.vector.tensor_copy(out=ones1_sb, in_=ones1f)
        zscr = perm.tile([P, IBS], f32, name="zscr", tag="zscr")
        nc.vector.memset(zscr, 0.0)
        zr = zscr.bitcast(f32r)
        eps_sb = perm.tile([8, 1], f32, name="eps", tag="eps")
        nc.vector.memset(eps_sb, EPS)

        # pvec columns per chunk: 0=gamma 1=beta 2=bg (bg pre-scaled x256)
        pvec_sb = perm.tile([P, NCK, 3], f32, name="pvec", tag="pvec")
        nc.gpsimd.dma_start(out=pvec_sb, in_=pvec_d.ap().rearrange("c p v -> p c v"))
        gamma_sb = [pvec_sb[:, ck, 0:1] for ck in range(NCK)]
        beta_sb = [pvec_sb[:, ck, 1:2] for ck in range(NCK)]
        bg_sb = [pvec_sb[:, ck, 2:3] for ck in range(NCK)]

        G_sb = perm.tile([P, NKP, 2, QPIX], f8, name="G", tag="G")
        vot_sb = perm.tile([P, NJP, 2, C], f8, name="vot", tag="vot")
        xt_all = perm.tile([P, NIB * NCK, C], f32, name="xt_all", tag="xt_all")
        svo_bc = perm.tile([P, C], f32, name="svo_bc", tag="svo_bc")
        # fp8 GN shift (x SHF_S), DoubleRow rhs layout [p, ckp, two]
        s8 = perm.tile([P, NKP, 2, 16], f8, name="s8", tag="s8")
        # stats scratch for the ACT chunks
        scrA = perm.tile([P, HW], f8, name="scrA", tag="scrA")

        with tc.tile_pool(name="wts", bufs=1) as wts, \
             tc.tile_pool(name="psA", bufs=1, space="PSUM") as psA:
            wkqt_sb = wts.tile([P, NKP, 2, C], f8, name="wkqt", tag="wkqt")
            nc.scalar.dma_start(out=wkqt_sb, in_=wkqt_d.ap())
            wovt_sb = wts.tile([P, NKP, 2, C], f8, name="wovt", tag="wovt")
            nc.sync.dma_start(out=wovt_sb, in_=wovt_d.ap())

            # PE warmup: f32r matmuls on zeros keep the HAM activity window
            # busy while x streams in and stats run, so real matmuls hit
            # 2.4 GHz.
            def warm_mms(n, tag):
                pw = psA.tile([P, IBS], f32, name=f"warm{tag}", tag="gn", bufs=1)
                for _ in range(n):
                    nc.tensor.matmul(pw, zr[:, 0:P], zr, start=True, stop=True)

            warm_mms(12, "w1")

            # ---- GroupNorm stats, split by engine per chunk ----
            # cmall[:, ck] = per-channel (mean, E[x^2])
            cmall = gnp.tile([P, NCK, 2], f32, name="cmall", tag="cmall",
                             bufs=1)
            for ck, eng in ((0, "dve"), (1, "dve"), (2, "act"), (3, "dve")):
                xc = x8[:, ck // 2, ck % 2, :]
                if eng == "dve":
                    stats = gnp.tile([P, NSUB, 6], f32, name="stats", tag="stats")
                    for s in range(NSUB):
                        nc.vector.bn_stats(out=stats[:, s, :],
                                           in_=xc[:, s * 512:(s + 1) * 512])
                    mv = gnp.tile([P, 2], f32, name="mv", tag="mv")
                    nc.vector.bn_aggr(out=mv, in_=stats)
                    # (mean, var) -> (mean, E[x^2])
                    nc.scalar.copy(out=cmall[:, ck, 0:1], in_=mv[:, 0:1])
                    nc.vector.scalar_tensor_tensor(
                        out=cmall[:, ck, 1:2], in0=mv[:, 0:1], scalar=mv[:, 0:1],
                        in1=mv[:, 1:2], op0=OP.mult, op1=OP.add)
                else:
                    pp = gnp.tile([P, 2, 2], f32, name="pp", tag="pp")
                    for h in range(2):
                        xh = xc[:, h * (HW // 2):(h + 1) * (HW // 2)]
                        nc.scalar.activation(out=scrA[:, 0:HW // 2], in_=xh,
                                             func=AF.Square,
                                             accum_out=pp[:, h, 1:2])
                        nc.scalar.activation(out=scrA[:, 0:HW // 2], in_=xh,
                                             func=AF.Identity,
                                             accum_out=pp[:, h, 0:1])
                    nc.vector.tensor_add(cmall[:, ck, :], pp[:, 0, :], pp[:, 1, :])
                    nc.vector.tensor_scalar_mul(cmall[:, ck, :],
                                                cmall[:, ck, :], 1.0 / HW)
                warm_mms(7, f"wgn{ck}")

            warm_mms(10, "waff")

            # ---- affine params, per chunk-pair (pair0 overlaps c3 stats) ----
            scl_all = gnp.tile([P, NCK], f32, name="scl_all", tag="scl_all",
                               bufs=1)
            shf_all = gnp.tile([P, NCK], f32, name="shf_all", tag="shf_all",
                               bufs=1)
            rscl_all = gnp.tile([P, NCK], f32, name="rscl_all", tag="rscl_all",
                                bufs=1)
            scale_sb = [scl_all[:, ck:ck + 1] for ck in range(NCK)]
            for kp in range(NKP):
                sl = slice(2 * kp, 2 * kp + 2)
                # aggregate to 8 groups x 2 chunks: (gmean, gE2)
                pg8 = psA.tile([8, 2, 2], f32, name="g8", tag="gn", bufs=1)
                nc.tensor.matmul(pg8, sel_sb, cmall[:, sl, :],
                                 start=True, stop=True)
                # critical path: var -> rstd -> broadcast -> scl -> W2
                gsq = gnp.tile([8, 2], f32, name="gsq", tag="gsq")
                nc.scalar.activation(out=gsq, in_=pg8[:, :, 0], func=AF.Square)
                grs = gnp.tile([8, 2], f32, name="grs", tag="grs")
                nc.vector.tensor_sub(grs, pg8[:, :, 1], gsq)
                nc.scalar.activation(out=grs, in_=grs, func=AF.Sqrt,
                                     bias=eps_sb, scale=1.0)
                nc.vector.reciprocal(out=grs, in_=grs)
                pbr = psA.tile([P, 2], f32, name="pbr", tag="g", bufs=2)
                nc.tensor.matmul(pbr, selt_sb, grs, start=True, stop=True)
                nc.vector.tensor_mul(scl_all[:, sl], pbr, pvec_sb[:, sl, 0])
                for ck in (2 * kp, 2 * kp + 1):
                    # W2 = D W in place (fp8), split DVE/ACT -- gates G/voT
                    wk_sl = wkqt_sb[:, ck // 2, ck % 2, :]
                    wo_sl = wovt_sb[:, ck // 2, ck % 2, :]
                    if ck % 2 == 0:
                        nc.vector.tensor_scalar_mul(wk_sl, wk_sl, scale_sb[ck])
                        nc.vector.tensor_scalar_mul(wo_sl, wo_sl, scale_sb[ck])
                    else:
                        nc.scalar.activation(out=wk_sl, in_=wk_sl,
                                             func=AF.Identity,
                                             scale=scale_sb[ck])
                        nc.scalar.activation(out=wo_sl, in_=wo_sl,
                                             func=AF.Identity,
                                             scale=scale_sb[ck])
                # off-critical: mean broadcast -> shift -> s8 (gates matvecs)
                gmn = gnp.tile([8, 2], f32, name="gmn", tag="gmn")
                nc.vector.tensor_copy(out=gmn, in_=pg8[:, :, 0])
                pbm = psA.tile([P, 2], f32, name="pbm", tag="g", bufs=2)
                nc.tensor.matmul(pbm, selt_sb, gmn, start=True, stop=True)
                tmp2 = gnp.tile([P, 2], f32, name="tmp2", tag="tmp2")
                nc.vector.tensor_mul(tmp2, pbm, scl_all[:, sl])
                nc.vector.tensor_sub(shf_all[:, sl], pvec_sb[:, sl, 1], tmp2)
                nc.vector.reciprocal(out=rscl_all[:, sl], in_=scl_all[:, sl])
                for ck in (2 * kp, 2 * kp + 1):
                    # s-tilde = s / d, x256, in fp8: feeding the matvecs with
                    # s/d against the D-scaled W2 recovers W^T s exactly.
                    nc.vector.tensor_scalar(
                        out=s8[:, ck // 2, ck % 2, 0:1],
                        in0=shf_all[:, ck:ck + 1],
                        scalar1=rscl_all[:, ck:ck + 1], scalar2=SHF_S,
                        op0=OP.mult, op1=OP.mult)

            # ---- s-fold matvecs (on W2 with s-tilde) ----
            # pgW[c] = 65536 * (Wkq^T s)[c]
            pgW = psA.tile([P, NCK], f32, name="pgW", tag="gn", bufs=1)
            for ci in range(NCK):
                for ckp in range(NKP):
                    nc.tensor.matmul(
                        pgW[:, ci:ci + 1],
                        wkqt_sb[:, ckp, :, ci * P:(ci + 1) * P],
                        s8[:, ckp, :, 0:1],
                        start=(ckp == 0), stop=(ckp == NKP - 1),
                        perf_mode=DR, skip_group_check=True)
            # dbG[c] = d_c * (256*bWs[c] + bg256[c]) -- G's fused bias
            dbG = gnp.tile([P, NCK], f32, name="dbG", tag="dbG", bufs=1)
            for ci in range(NCK):
                t0 = gnp.tile([P, 1], f32, name="t0", tag="t0")
                nc.vector.scalar_tensor_tensor(
                    out=t0, in0=pgW[:, ci:ci + 1], scalar=1.0 / SHF_S,
                    in1=bg_sb[ci], op0=OP.mult, op1=OP.add)
                nc.vector.tensor_mul(dbG[:, ci:ci + 1], t0, scale_sb[ci])

            # svo[o] = (s^T Wov)[o], on one partition, x4096-scaled
            psvo = psA.tile([1, C], f32, name="psvo", tag="gn", bufs=1)
            for ckp in range(NKP):
                nc.tensor.matmul(
                    psvo,
                    s8[:, ckp, :, 0:1],
                    wovt_sb[:, ckp, :, :],
                    start=(ckp == 0), stop=(ckp == NKP - 1),
                    perf_mode=DR, skip_group_check=True)
            svo_row = gnp.tile([1, C], f32r, name="svo_row", tag="svo_row")
            # psvo is x(SHF_S * WOV_S); xt_all is in raw residual units
            nc.scalar.activation(out=svo_row, in_=psvo, func=AF.Identity,
                                 scale=1.0 / (SHF_S * WOV_S))
            # broadcast svo to all partitions via K=1 outer product
            psvb = psA.tile([P, C], f32, name="psvb", tag="gn", bufs=1)
            nc.tensor.matmul(psvb, ones1_sb, svo_row,
                             start=True, stop=True)
            nc.vector.tensor_copy(out=svo_bc, in_=psvb)

            # ---- G2 = d * (W2^T x) + dbG,  fp8 DoubleRow ----
            for ib in range(NIB):
                for ci in range(NCK):
                    pg = psA.tile([P, IBS], f32, name="g", tag="g", bufs=2)
                    for ckp in range(NKP):
                        nc.tensor.matmul(
                            pg,
                            wkqt_sb[:, ckp, :, ci * P:(ci + 1) * P],
                            x8[:, ckp, :, ib * IBS:(ib + 1) * IBS],
                            start=(ckp == 0), stop=(ckp == NKP - 1),
                            perf_mode=DR)
                    gsl = G_sb[:, ci // 2, ci % 2, ib * IBS:(ib + 1) * IBS]
                    if ci % 2 == 0:
                        nc.scalar.activation(out=gsl, in_=pg,
                                             func=AF.Identity,
                                             bias=dbG[:, ci:ci + 1],
                                             scale=scale_sb[ci])
                    else:
                        nc.vector.tensor_scalar(
                            out=gsl, in0=pg,
                            scalar1=scale_sb[ci], scalar2=dbG[:, ci:ci + 1],
                            op0=OP.mult, op1=OP.add)

            # ---- voT = x^T (D Wov),  fp8 DoubleRow; svo lands in xt_all ----
            for jt in range(NJT):
                pv = psA.tile([P, C], f32, name="vt", tag="vt", bufs=5)
                for ckp in range(NKP):
                    nc.tensor.matmul(
                        pv,
                        x8[:, ckp, :, jt * P:(jt + 1) * P],
                        wovt_sb[:, ckp, :, :],
                        start=(ckp == 0), stop=(ckp == NKP - 1),
                        perf_mode=DR)
                if jt % 2 == 0:
                    nc.scalar.copy(out=vot_sb[:, jt // 2, jt % 2, :], in_=pv)
                else:
                    nc.vector.tensor_copy(out=vot_sb[:, jt // 2, jt % 2, :], in_=pv)

        # residual (transposed, host-folded) — needed only in the tail.
        # svo (the GN-shift term of voT) folds in here exactly: attention
        # rows sum to 1, so out_i picks up +svo once per row.
        nc.sync.dma_start(out=xt_all, in_=xt_d.ap().rearrange("(g p) o -> p g o", p=P))
        for g in range(NIB * NCK):
            nc.gpsimd.tensor_add(xt_all[:, g, :], xt_all[:, g, :], svo_bc)

        # ---- attention ----
        with tc.tile_pool(name="att", bufs=2) as att, \
             tc.tile_pool(name="psB", bufs=1, space="PSUM") as psB:
            oq = [nc.sync, nc.scalar, nc.gpsimd]
            pend_tail = [None]

            def make_tail(ib, pavs, racc):
                def tail():
                    # transposed rowsums: prT[:, s] = 16 * sum_p (racc0+racc1)
                    prT = psB.tile([P, NCK], f32, name="rT", tag="e", bufs=4)
                    for s in range(NCK):
                        for par in range(2):
                            nc.tensor.matmul(prT[:, s:s + 1],
                                             racc[par][:, s * P:(s + 1) * P],
                                             sixt_sb,
                                             start=(par == 0), stop=(par == 1),
                                             skip_group_check=True)
                    rT_sb = att.tile([P, NCK], f32, name="rT_sb", tag="rT_sb",
                                     bufs=2)
                    nc.vector.reciprocal_approx_fast(out=rT_sb, in_=prT)
                    for isub in range(NCK):
                        g = ib * NCK + isub
                        t = att.tile([P, C], f32, name="t_out", tag="t_out",
                                     bufs=4)
                        for h in range(2):
                            hs = slice(h * (C // 2), (h + 1) * (C // 2))
                            nc.vector.scalar_tensor_tensor(
                                out=t[:, hs], in0=pavs[isub][:, hs],
                                scalar=rT_sb[:, isub:isub + 1],
                                in1=xt_all[:, g, hs],
                                op0=OP.mult, op1=OP.add)
                            oq[(2 * isub + h) % 3].dma_start(
                                out=out_r[g][:, hs], in_=t[:, hs])
                return tail

            for ib in range(NIB):
                pavs = [psB.tile([P, C], f32, name=f"av{ok}", tag="av", bufs=4)
                        for ok in range(NCK)]
                # rowsum accumulators: parity 0 on DVE, parity 1 on GPSIMD
                racc = [att.tile([P, IBS], f32, name=f"racc{par}",
                                 tag=f"racc{par}", bufs=2) for par in range(2)]
                reng = [nc.vector, nc.gpsimd]

                def av_group(jp, e_t):
                    for isub in range(NCK):
                        nc.tensor.matmul(
                            pavs[isub],
                            e_t[:, :, isub * P:(isub + 1) * P],
                            vot_sb[:, jp, :, :],
                            start=(jp == 0), stop=(jp == NJP - 1),
                            perf_mode=DR, skip_group_check=True)

                pends = []  # (jp, e_pair) with exp in flight; av 2 iters later
                for jp in range(NJP):
                    pes = []
                    for par in range(2):
                        jt = jp * 2 + par
                        pe = psB.tile([P, IBS], f32, name="e", tag="e", bufs=4)
                        for ckp in range(NKP):
                            nc.tensor.matmul(
                                pe,
                                x8[:, ckp, :, jt * P:(jt + 1) * P],
                                G_sb[:, ckp, :, ib * IBS:(ib + 1) * IBS],
                                start=(ckp == 0), stop=(ckp == NKP - 1),
                                perf_mode=DR)
                        pes.append(pe)
                    if len(pends) >= 2:
                        av_group(*pends.pop(0))
                    if jp == 1 and pend_tail[0] is not None:
                        pend_tail[0]()
                        pend_tail[0] = None
                    e_pair = att.tile([P, 2, IBS], f8, name="e_pair",
                                      tag="e_pair", bufs=4)
                    for par in range(2):
                        nc.scalar.activation(out=e_pair[:, par, :], in_=pes[par],
                                             func=AF.Exp, scale=1.0 / WKQ_S)
                        if jp == 0:
                            reng[par].tensor_copy(out=racc[par],
                                                  in_=e_pair[:, par, :])
                        else:
                            reng[par].tensor_add(racc[par], racc[par],
                                                 e_pair[:, par, :])
                    pends.append((jp, e_pair))
                for p_ in pends:
                    av_group(*p_)
                pend_tail[0] = make_tail(ib, pavs, racc)
            pend_tail[0]()

    nc.compile()
    return nc


def _get_nc():
    if "nc" not in _CACHE:
        _CACHE["nc"] = _build_nc()
    return _CACHE["nc"]


def _to_dr_layout(w):
    # [c', m] -> [k, ckp, two, m] with c' = ckp*256 + two*128 + k
    return np.ascontiguousarray(
        w.reshape(NKP, 2, P, C).transpose(2, 0, 1, 3))


def make_in_maps(**inputs):
    x = np.asarray(inputs["x"], np.float64).reshape(B, C, HW)
    gamma = np.asarray(inputs["gamma"], np.float64)
    beta = np.asarray(inputs["beta"], np.float64)
    wq = np.asarray(inputs["wq"], np.float64)
    bq = np.asarray(inputs["bq"], np.float64)
    wk = np.asarray(inputs["wk"], np.float64)
    wv = np.asarray(inputs["wv"], np.float64)
    bv = np.asarray(inputs["bv"], np.float64)
    wo = np.asarray(inputs["wo"], np.float64)
    bo = np.asarray(inputs["bo"], np.float64)
    cs = 1.0 / np.sqrt(C)
    f8 = ml_dtypes.float8_e4m3

    wkqt = _to_dr_layout((wq.T @ wk) * cs * WKQ_S).astype(f8)    # [k,ckp,2,ci]
    bg = (wk.T @ (bq * cs)) * WKQ_S
    wovt = _to_dr_layout((wv.T @ wo.T) * WOV_S).astype(f8)       # [k,ckp,2,o]
    addc = (wo @ bv + bo)
    pvec = np.ascontiguousarray(
        np.stack([gamma.reshape(NCK, P), beta.reshape(NCK, P),
                  bg.reshape(NCK, P)], axis=2).astype(np.float32))

    in_maps = []
    for core in range(8):
        b, q = divmod(core, 4)
        xb = np.roll(x[b], -q * QPIX, axis=1)
        xt = np.ascontiguousarray(xb[:, :QPIX].T + addc[None, :]).astype(np.float32)
        in_maps.append({
            "x": np.ascontiguousarray(xb).astype(f8),
            "wkqt": wkqt, "wovt": wovt, "pvec": pvec, "xt": xt,
        })
    return in_maps


def assemble(results):
    out = np.empty((B, C, HW), np.float32)
    for core in range(8):
        b, q = divmod(core, 4)
        out[b][:, q * QPIX:(q + 1) * QPIX] = results[core]["out"].T
    return out.reshape(B, C, H, W)


def kernel(**inputs):
    from concourse.bass_utils import run_bass_kernel_spmd
    nc = _get_nc()
    in_maps = make_in_maps(**inputs)
    res = run_bass_kernel_spmd(nc, in_maps, core_ids=list(range(8)))
    return assemble(res.results)
